# revision 4
# baseline (speedup 1.0000x reference)
"""NSMCell message-passing kernel for 8 Trainium2 NeuronCores.

Contract: kernel(**inputs) takes the FULL unsharded inputs (numpy/jax arrays)
and returns the FULL (N,) float32 output, matching reference.reference().

Math restructuring (exact, up to float assoc.):
  edge path:  t_e = w_rel . elu((i_b (*) a_e) @ W_edge),  b = edge_batch[e].
              Fold the gating into the attrs on host: a'_e = i_b (*) a_e,
              so ONE global stationary W_edge serves every edge - no graph
              boundaries on device, no edge sorting, cores take equal slabs.
  node path:  s_n = w_node . elu(sum_p (sim_bp * i_b (*) attr_np) @ W_props[p])
              with the (sim*i) gating likewise folded into attrs on host.
  host epilogue (O(N+E) scalar work): scatter-add dist[src]*t into nodes by
  dst, two segment softmaxes over graphs, final mix by relation_similarity.

Device pipeline per 1536-col z-tile (cols = edges or nodes, H=128 on
partitions):
  PE   : z = W^T @ a'            (3x512-col fp16 mains, f32 PSUM)
  ACT  : e = exp(z)              (fp16; saturates to inf for z>11, handled)
  DVE  : psi = relu(z) + min(e,1) - 1 = elu(z)   (one fused 4-op custom op)
  PE   : reduce matmul (emitted D tiles late so the PE never stalls on DVE):
         stationary = one-hot column j carrying w_rel; accumulates row j of a
         (32,512) PSUM bank; 32 chunks share a bank before a tiny ACT evac.
The w.elu dot therefore costs 1 matmul per 512 cols instead of the per-128
LoadStationary matvec storm, and no per-graph weight tables are streamed.
"""

import os
import sys
import types
from collections import deque

import numpy as np

# ---------------------------------------------------------------------------
# problem constants (hardcoded per contract)
N, P, H, E, B = 100000, 4, 128, 1000000, 64
NCORES = 8
EC = E // NCORES            # 125000 edges per core (exact equal slabs)
NC = N // NCORES            # 12500 nodes per core
ZT = 1536                   # edge z-tile cols (3 PSUM banks)
EC_PAD = (EC + ZT - 1) // ZT * ZT          # 125952 = 82*1536 exactly
N_ETILES = EC_PAD // ZT                    # 82
NT = (NC + 511) // 512                     # 25 node tiles of 512
JROWS = 32                  # t-accumulator PSUM rows (one-hot stationary set)
N_RED_E = EC_PAD // 512     # 246 edge reduce matmuls
TG = (N_RED_E + JROWS - 1) // JROWS        # 8 t-bank generations
RED_DELAY = 2               # emit tile k's reduces after tile k+RED_DELAY


# ---------------------------------------------------------------------------
def _install_ntff_hook():
    """Allow BASS_TRACE=1 profiling under axon (test.py); harmless otherwise."""
    try:
        from antenv.axon_hooks import get_axon_ntff_profile_hook  # noqa: F401
        return
    except ImportError:
        pass
    try:
        from trn_agent_boot.trn_boot import _ntff_profile_via_ctypes
        hook = _ntff_profile_via_ctypes("/opt/axon/libaxon_pjrt.so")
    except Exception:
        hook = None
    mod = types.ModuleType("antenv.axon_hooks")
    _state = {"hook": hook}
    mod.get_axon_ntff_profile_hook = lambda: _state["hook"]
    mod.set_axon_ntff_profile_hook = lambda h: _state.__setitem__("hook", h)
    sys.modules["antenv.axon_hooks"] = mod
    try:
        import antenv
        antenv.axon_hooks = mod
    except ImportError:
        pass


def _make_elu_op():
    """Register custom DVE op: out = s0 * (relu(in0) + min(in1, 1) - 1)
    (= s0 * elu(in0) when in1 == exp(in0); s0 is a per-partition weight AP).
    Runtime registration: append to dve_ops.OPS."""
    from concourse import dve_ops
    from concourse.dve_spec import (Spec, Src0, Src1, C0, One, relu, minn,
                                    lower)
    from concourse.dve_uop import DveOpSpec

    name = "WELU_FROM_EXP_ANT"
    for op in dve_ops.OPS:
        if op.name == name:
            return op
    spec = Spec(
        body=(relu(Src0) + minn(Src1, One) - One) * C0,
        reference=lambda in0, in1, s0, s1, imm2: (
            (np.maximum(np.nan_to_num(in0, nan=0.0), 0)
             + np.minimum(in1, np.float32(1.0))
             - np.float32(1.0)) * s0
        ).astype(np.float32),
    )
    row = dve_ops._CUSTOM_DVE_ROW_BASE + len(dve_ops.OPS)
    assert row < 0x20
    shas = {}
    for ver in ("v3", "v4"):
        shas[ver] = DveOpSpec(
            name=name, opcode=row, uops=lower(spec, ver=ver), rd1_en=True
        ).sha(ver)
    op = dve_ops.DveOp(name, spec, subdim=False, uops_sha=shas)
    dve_ops.OPS.append(op)
    dve_ops.CUSTOM_DVE_SPECS[name] = spec
    dve_ops._SUB_OPCODE_FOR_NAME[name] = row
    return op


# ---------------------------------------------------------------------------
def _build_program():
    """Single SPMD bass program; every core runs an identical flat stream."""
    import concourse.tile as tile
    from concourse import bacc
    import concourse.mybir as mybir

    f32 = mybir.dt.float32
    f16 = mybir.dt.float16
    Exp = mybir.ActivationFunctionType.Exp
    elu_op = _make_elu_op()

    nc = bacc.Bacc("TRN2", target_bir_lowering=False, debug=False,
                   num_devices=NCORES)

    ea_in = nc.dram_tensor("ea_t", [H, EC_PAD], f16, kind="ExternalInput")
    na_in = nc.dram_tensor("na_t", [H, NT * P * 512], f16,
                           kind="ExternalInput")
    we_in = nc.dram_tensor("we_t", [H, H], f16, kind="ExternalInput")
    wp_in = nc.dram_tensor("wp_t", [H, P, H], f16, kind="ExternalInput")
    wohe_in = nc.dram_tensor("woh_e", [H, JROWS, JROWS], f16,
                             kind="ExternalInput")
    wohn_in = nc.dram_tensor("woh_n", [H, JROWS, JROWS], f16,
                             kind="ExternalInput")
    # t_out[j, g*512 + c] = t of edge ((g*32 + j)*512 + c)
    t_out = nc.dram_tensor("t_out", [JROWS, TG * 512], f32,
                           kind="ExternalOutput")
    # s_out[j, c] = s of node j*512 + c  (single generation, NT rows used)
    s_out = nc.dram_tensor("s_out", [JROWS, 512], f32, kind="ExternalOutput")

    # edge DMA pieces (col ranges); first piece split for a fast launch
    epieces = [(0, 1024), (1024, 3072)]
    off = 4096
    while off < EC_PAD:
        w = min(4096, EC_PAD - off)
        epieces.append((off, w))
        off += w
    # node DMA pieces over the NT*P*512 column stream
    npieces = []
    off = 0
    ncols_total = NT * P * 512
    while off < ncols_total:
        w = min(4096, ncols_total - off)
        npieces.append((off, w))
        off += w

    with tile.TileContext(nc) as tc:
        with (
            tc.tile_pool(name="consts", bufs=1) as cpool,
            tc.tile_pool(name="ework", bufs=6) as epool,
            tc.tile_pool(name="nwork", bufs=3) as npool,
            tc.tile_pool(name="expw", bufs=3) as xpool,
            tc.tile_pool(name="psi", bufs=4) as ppool,
            tc.tile_pool(name="outs", bufs=2) as opool,
            tc.tile_pool(name="zpsum", bufs=2, space="PSUM") as zpool,
            tc.tile_pool(name="tpsum", bufs=1, space="PSUM") as tpool,
            tc.tile_pool(name="spsum", bufs=1, space="PSUM") as spool,
        ):
            we_sb = cpool.tile([H, H], f16)
            nc.sync.dma_start(we_sb[:], we_in.ap())
            wohe_sb = cpool.tile([H, JROWS, JROWS], f16)
            nc.sync.dma_start(wohe_sb[:], wohe_in.ap())
            wp_sb = cpool.tile([H, P, H], f16)
            nc.sync.dma_start(wp_sb[:], wp_in.ap())
            wohn_sb = cpool.tile([H, JROWS, JROWS], f16)
            nc.sync.dma_start(wohn_sb[:], wohn_in.ap())

            ea_tiles = {}   # piece start -> tile
            na_tiles = {}
            eni, nni = 0, 0  # next piece index to DMA

            # reduce state (edge)
            state = {"tacc": None, "tj": 0, "tgen": 0, "red": 0,
                     "sacc": None, "sj": 0}
            pend_e = deque()   # psi tiles awaiting their reduce matmuls
            pend_n = deque()

            def flush_edge_reduce():
                psi = pend_e.popleft()
                for h in range(ZT // 512):
                    if state["tacc"] is None:
                        state["tacc"] = tpool.tile([JROWS, 512], f32,
                                                   name="tacc", tag="tacc")
                        state["tj"] = 0
                    tj = state["tj"]
                    last = (tj == JROWS - 1) or (state["red"] == N_RED_E - 1)
                    nc.tensor.matmul(
                        state["tacc"][:], wohe_sb[:, tj, :],
                        psi[:, h * 512:h * 512 + 512],
                        start=(tj == 0), stop=last,
                        skip_group_check=True,
                    )
                    state["tj"] += 1
                    state["red"] += 1
                    if last:
                        tsb = opool.tile([JROWS, 512], f32, tag="osb")
                        nc.scalar.copy(tsb[:], state["tacc"][:])
                        g = state["tgen"]
                        nc.sync.dma_start(
                            t_out.ap()[:, g * 512:g * 512 + 512], tsb[:])
                        state["tacc"] = None
                        state["tgen"] += 1

            def flush_node_reduce():
                psi, m = pend_n.popleft()
                if state["sacc"] is None:
                    state["sacc"] = spool.tile([JROWS, 512], f32, name="sacc",
                                               tag="sacc")
                nc.tensor.matmul(
                    state["sacc"][:], wohn_sb[:, m, :], psi[:, 0:512],
                    start=(m == 0), stop=(m == NT - 1),
                    skip_group_check=True,
                )
                if m == NT - 1:
                    ssb = opool.tile([JROWS, 512], f32, tag="osb")
                    nc.scalar.copy(ssb[:], state["sacc"][:])
                    nc.sync.dma_start(s_out.ap()[:], ssb[:])

            def emit_node_tile(m):
                nonlocal nni
                c0 = m * P * 512
                while nni < len(npieces) and npieces[nni][0] < c0 + P * 512:
                    p0, pw = npieces[nni]
                    pt = npool.tile([H, 4096], f16, tag="na")
                    nc.sync.dma_start(pt[:, :pw], na_in.ap()[:, p0:p0 + pw])
                    na_tiles[p0] = pt
                    nni += 1
                z = zpool.tile([H, ZT], f32, tag="z")
                for p in range(P):
                    c = c0 + p * 512
                    q0 = max(q for q in na_tiles if q <= c)
                    pt = na_tiles[q0]
                    nc.tensor.matmul(
                        z[:, 0:512], wp_sb[:, p, :], pt[:, c - q0:c - q0 + 512],
                        start=(p == 0), stop=(p == P - 1),
                    )
                ex = xpool.tile([H, ZT], f16, tag="ex")
                nc.scalar.activation(ex[:, 0:512], z[:, 0:512], Exp)
                psi = ppool.tile([H, ZT], f16, tag="psi")
                nc.vector._custom_dve(elu_op, out=psi[:, 0:512],
                                      in0=z[:, 0:512], in1=ex[:, 0:512],
                                      s0=1.0)
                pend_n.append((psi, m))

            node_next = 0
            for k in range(N_ETILES):
                c0 = k * ZT
                while eni < len(epieces) and epieces[eni][0] < c0 + ZT:
                    p0, pw = epieces[eni]
                    pt = epool.tile([H, 4096], f16, tag="ea")
                    nc.sync.dma_start(pt[:, :pw], ea_in.ap()[:, p0:p0 + pw])
                    ea_tiles[p0] = pt
                    eni += 1
                z = zpool.tile([H, ZT], f32, tag="z")
                for h in range(ZT // 512):
                    c = c0 + h * 512
                    q0 = max(q for q in ea_tiles if q <= c)
                    pt = ea_tiles[q0]
                    nc.tensor.matmul(
                        z[:, h * 512:h * 512 + 512], we_sb[:],
                        pt[:, c - q0:c - q0 + 512],
                        start=True, stop=True,
                    )
                ex = xpool.tile([H, ZT], f16, tag="ex")
                nc.scalar.activation(ex[:], z[:], Exp)
                psi = ppool.tile([H, ZT], f16, tag="psi")
                nc.vector._custom_dve(elu_op, out=psi[:], in0=z[:],
                                      in1=ex[:], s0=1.0)
                pend_e.append(psi)
                if len(pend_e) > RED_DELAY:
                    flush_edge_reduce()
                # interleave node tiles proportionally into the edge stream
                m_target = (k + 1) * NT // N_ETILES
                while node_next < min(m_target, NT):
                    if len(pend_n) >= 1:
                        flush_node_reduce()
                    emit_node_tile(node_next)
                    node_next += 1
            while node_next < NT:
                if len(pend_n) >= 1:
                    flush_node_reduce()
                emit_node_tile(node_next)
                node_next += 1
            while pend_e:
                flush_edge_reduce()
            while pend_n:
                flush_node_reduce()

    nc.compile()
    return nc


# ---------------------------------------------------------------------------
def kernel(node_attrs, edge_attrs, instruction_batch, distribution,
           node_prop_similarities, relation_similarity,
           W_props, W_edge, w_node_score, w_rel_score,
           edge_indices, node_indices, edge_batch_indices):
    _install_ntff_hook()
    from concourse import bass_utils

    f16 = np.float16
    ea = np.asarray(edge_attrs, np.float32)
    na = np.asarray(node_attrs, np.float32)
    ib = np.asarray(instruction_batch, np.float32)
    dist = np.asarray(distribution, np.float32)
    nps = np.asarray(node_prop_similarities, np.float32)
    rs = np.asarray(relation_similarity, np.float32)
    Wp = np.asarray(W_props, np.float32)
    We = np.asarray(W_edge, np.float32)
    wn = np.asarray(w_node_score, np.float32)
    wr = np.asarray(w_rel_score, np.float32)
    ei = np.asarray(edge_indices).astype(np.int64)
    ni = np.asarray(node_indices).astype(np.int64)
    ebi = np.asarray(edge_batch_indices).astype(np.int64)
    src, dst = ei[0], ei[1]

    # ---- host prep: fold gatings into attrs, shard into equal slabs ----
    ea_t = np.zeros((NCORES, H, EC_PAD), f16)
    for c in range(NCORES):
        sl = slice(c * EC, (c + 1) * EC)
        ap = ea[sl] * ib[ebi[sl]]                      # (EC, H) f32
        ea_t[c, :, :EC] = ap.astype(f16).T

    na_t = np.zeros((NCORES, H, NT * P * 512), f16)
    for c in range(NCORES):
        sl = slice(c * NC, (c + 1) * NC)
        gate = nps[ni[sl]][:, :, None] * ib[ni[sl]][:, None, :]  # (NC,P,H)
        buf = np.zeros((NT * 512, P, H), f16)
        buf[:NC] = (na[sl] * gate).astype(f16)
        # (NT,512,P,H) -> (H, NT, P, 512) -> (H, NT*P*512)
        na_t[c] = np.ascontiguousarray(
            buf.reshape(NT, 512, P, H).transpose(3, 0, 2, 1)
        ).reshape(H, NT * P * 512)

    we_t = We.astype(f16)                              # (H[h], H[k])
    wp_t = np.ascontiguousarray(Wp.transpose(1, 0, 2)).astype(f16)
    woh_e = np.zeros((H, JROWS, JROWS), f16)
    woh_e[:, np.arange(JROWS), np.arange(JROWS)] = wr[:, None]
    woh_n = np.zeros((H, JROWS, JROWS), f16)
    woh_n[:, np.arange(JROWS), np.arange(JROWS)] = wn[:, None]

    nc = _build_program()

    in_maps = []
    for c in range(NCORES):
        in_maps.append({
            "ea_t": ea_t[c],
            "na_t": na_t[c],
            "we_t": we_t,
            "wp_t": wp_t,
            "woh_e": woh_e,
            "woh_n": woh_n,
        })

    res = bass_utils.run_bass_kernel_spmd(
        nc, in_maps, core_ids=list(range(NCORES)),
        trace=bool(os.environ.get("BASS_TRACE")),
        tmpdir=os.environ.get("KERNEL_TRACE_DIR") or None,
    )
    kernel.last_results = res  # for test.py profiling introspection

    # ---- host epilogue ----
    t_full = np.empty(E, np.float64)
    s_full = np.empty(N, np.float64)
    for c in range(NCORES):
        td = np.asarray(res.results[c]["t_out"], np.float64)  # (32, TG*512)
        # t[q*512+r] = td[q%32, (q//32)*512 + r]
        tc_ = td.reshape(JROWS, TG, 512).transpose(1, 0, 2).reshape(-1)[:EC]
        t_full[c * EC:(c + 1) * EC] = tc_
        sd = np.asarray(res.results[c]["s_out"], np.float64)  # (32, 512)
        s_full[c * NC:(c + 1) * NC] = sd.reshape(-1)[:NC]

    acc = np.bincount(dst, weights=dist[src].astype(np.float64) * t_full,
                      minlength=N)

    def seg_softmax(x):
        m = np.full(B, -np.inf)
        np.maximum.at(m, ni, x)
        e = np.exp(x - m[ni])
        ssum = np.zeros(B, np.float64)
        np.add.at(ssum, ni, e)
        return e / ssum[ni]

    next_rel = seg_softmax(acc)
    next_states = seg_softmax(s_full)
    rsn = rs[ni].astype(np.float64)
    out = rsn * next_rel + (1.0 - rsn) * next_states
    return out.astype(np.float32)


# revision 6
# speedup vs baseline: 1.0223x; 1.0223x over previous
"""NSMCell message-passing kernel for 8 Trainium2 NeuronCores.

Contract: kernel(**inputs) takes the FULL unsharded inputs (numpy/jax arrays)
and returns the FULL (N,) float32 output, matching reference.reference().

Math restructuring (exact, up to float assoc.):
  edge path:  t_e = w_rel . elu((i_b (*) a_e) @ W_edge),  b = edge_batch[e].
              Fold the gating into the attrs on host: a'_e = i_b (*) a_e,
              so ONE global stationary W_edge serves every edge - no graph
              boundaries on device, no edge sorting, cores take equal slabs.
  node path:  s_n = w_node . elu(sum_p (sim_bp * i_b (*) attr_np) @ W_props[p])
              with the (sim*i) gating likewise folded into attrs on host.
  host epilogue (O(N+E) scalar work): scatter-add dist[src]*t into nodes by
  dst, two segment softmaxes over graphs, final mix by relation_similarity.

Device pipeline per 1024-col z-tile (cols = edges or nodes, H=128 on
partitions):
  PE   : z = W^T @ a'            (2x512-col fp16 mains, f32 PSUM)
  ACT  : e = exp(z)              (fp16; saturates to inf for z>11, handled)
  DVE  : psi = relu(z) + min(e,1) - 1 = elu(z)   (one fused 4-op custom op)
  PE   : reduce matmul (emitted 2 tiles late so the PE never stalls on DVE):
         stationary = one-hot column j carrying w_rel (or w_node for node
         chunks); accumulates row j of a (32,512) PSUM bank. Edge and node
         chunks share ONE unified reduce stream / bank set, so PSUM fits
         3 z-buffers (6 banks) + 2 t-banks = 8 exactly.
The w.elu dot therefore costs 1 matmul per 512 cols instead of the per-128
LoadStationary matvec storm, and no per-graph weight tables are streamed.
_schedule() replays the emission order on the host to un-weave t_out.
"""

import os
import sys
import types
from collections import deque

import numpy as np

# ---------------------------------------------------------------------------
# problem constants (hardcoded per contract)
N, P, H, E, B = 100000, 4, 128, 1000000, 64
NCORES = 8
EC = E // NCORES            # 125000 edges per core (exact equal slabs)
NC = N // NCORES            # 12500 nodes per core
ZT = 1024                   # edge z-tile cols (2 PSUM banks)
EC_PAD = (EC + ZT - 1) // ZT * ZT          # 125952
N_ETILES = EC_PAD // ZT                    # 123
NT = (NC + 511) // 512                     # 25 node tiles of 512
JROWS = 32                  # t-accumulator PSUM rows (one-hot stationary set)
N_RED = EC_PAD // 512 + NT  # 271 reduce chunks (edge + node unified stream)
TG = (N_RED + JROWS - 1) // JROWS          # 9 t-bank generations
RED_DELAY = 2               # emit a tile's reduces two tiles later


def _schedule():
    """Replay of the build-time emission order.

    Returns (tiles, chunks): tiles = list of ("e", k) / ("n", m) in tile
    emission order; chunks = list of ("e", edge_512_block) / ("n", m) in
    reduce-chunk order (the order chunks enter the shared t-bank stream)."""
    tiles = []
    node_next = 0
    for k in range(N_ETILES):
        tiles.append(("e", k))
        m_target = (k + 1) * NT // N_ETILES
        while node_next < min(m_target, NT):
            tiles.append(("n", node_next))
            node_next += 1
    while node_next < NT:
        tiles.append(("n", node_next))
        node_next += 1
    chunks = []
    pend = deque()
    for kind, idx in tiles:
        if kind == "e":
            pend.append([("e", idx * 2), ("e", idx * 2 + 1)])
        else:
            pend.append([("n", idx)])
        if len(pend) > RED_DELAY:
            chunks.extend(pend.popleft())
    while pend:
        chunks.extend(pend.popleft())
    assert len(chunks) == N_RED
    return tiles, chunks


# ---------------------------------------------------------------------------
def _install_ntff_hook():
    """Allow BASS_TRACE=1 profiling under axon (test.py); harmless otherwise."""
    try:
        from antenv.axon_hooks import get_axon_ntff_profile_hook  # noqa: F401
        return
    except ImportError:
        pass
    try:
        from trn_agent_boot.trn_boot import _ntff_profile_via_ctypes
        hook = _ntff_profile_via_ctypes("/opt/axon/libaxon_pjrt.so")
    except Exception:
        hook = None
    mod = types.ModuleType("antenv.axon_hooks")
    _state = {"hook": hook}
    mod.get_axon_ntff_profile_hook = lambda: _state["hook"]
    mod.set_axon_ntff_profile_hook = lambda h: _state.__setitem__("hook", h)
    sys.modules["antenv.axon_hooks"] = mod
    try:
        import antenv
        antenv.axon_hooks = mod
    except ImportError:
        pass


def _make_elu_op():
    """Register custom DVE op: out = s0 * (relu(in0) + min(in1, 1) - 1)
    (= s0 * elu(in0) when in1 == exp(in0)).
    Runtime registration: append to dve_ops.OPS."""
    from concourse import dve_ops
    from concourse.dve_spec import (Spec, Src0, Src1, C0, One, relu, minn,
                                    lower)
    from concourse.dve_uop import DveOpSpec

    name = "WELU_FROM_EXP_ANT"
    for op in dve_ops.OPS:
        if op.name == name:
            return op
    spec = Spec(
        body=(relu(Src0) + minn(Src1, One) - One) * C0,
        reference=lambda in0, in1, s0, s1, imm2: (
            (np.maximum(np.nan_to_num(in0, nan=0.0), 0)
             + np.minimum(in1, np.float32(1.0))
             - np.float32(1.0)) * s0
        ).astype(np.float32),
    )
    row = dve_ops._CUSTOM_DVE_ROW_BASE + len(dve_ops.OPS)
    assert row < 0x20
    shas = {}
    for ver in ("v3", "v4"):
        shas[ver] = DveOpSpec(
            name=name, opcode=row, uops=lower(spec, ver=ver), rd1_en=True
        ).sha(ver)
    op = dve_ops.DveOp(name, spec, subdim=False, uops_sha=shas)
    dve_ops.OPS.append(op)
    dve_ops.CUSTOM_DVE_SPECS[name] = spec
    dve_ops._SUB_OPCODE_FOR_NAME[name] = row
    return op


# ---------------------------------------------------------------------------
def _build_program():
    """Single SPMD bass program; every core runs an identical flat stream."""
    import concourse.tile as tile
    from concourse import bacc
    import concourse.mybir as mybir

    f32 = mybir.dt.float32
    f16 = mybir.dt.float16
    Exp = mybir.ActivationFunctionType.Exp
    elu_op = _make_elu_op()

    nc = bacc.Bacc("TRN2", target_bir_lowering=False, debug=False,
                   num_devices=NCORES)

    ea_in = nc.dram_tensor("ea_t", [H, EC_PAD], f16, kind="ExternalInput")
    na_in = nc.dram_tensor("na_t", [H, NT * P * 512], f16,
                           kind="ExternalInput")
    we_in = nc.dram_tensor("we_t", [H, H], f16, kind="ExternalInput")
    wp_in = nc.dram_tensor("wp_t", [H, P, H], f16, kind="ExternalInput")
    wohe_in = nc.dram_tensor("woh_e", [H, JROWS, JROWS], f16,
                             kind="ExternalInput")
    wohn_in = nc.dram_tensor("woh_n", [H, JROWS, JROWS], f16,
                             kind="ExternalInput")
    # t_out[j, g*512 + c] = value of reduce chunk (g*32 + j), col c; chunk
    # order is _schedule()[1] (mixed edge/node blocks of 512)
    t_out = nc.dram_tensor("t_out", [JROWS, TG * 512], f32,
                           kind="ExternalOutput")

    # edge DMA pieces (col ranges); first piece split for a fast launch
    epieces = [(0, 1024), (1024, 3072)]
    off = 4096
    while off < EC_PAD:
        w = min(4096, EC_PAD - off)
        epieces.append((off, w))
        off += w
    # node DMA pieces over the NT*P*512 column stream
    npieces = []
    off = 0
    ncols_total = NT * P * 512
    while off < ncols_total:
        w = min(4096, ncols_total - off)
        npieces.append((off, w))
        off += w

    tiles, _ = _schedule()

    with tile.TileContext(nc) as tc:
        with (
            tc.tile_pool(name="consts", bufs=1) as cpool,
            tc.tile_pool(name="ework", bufs=6) as epool,
            tc.tile_pool(name="nwork", bufs=3) as npool,
            tc.tile_pool(name="expw", bufs=4) as xpool,
            tc.tile_pool(name="psi", bufs=5) as ppool,
            tc.tile_pool(name="outs", bufs=2) as opool,
            tc.tile_pool(name="zpsum", bufs=3, space="PSUM") as zpool,
            tc.tile_pool(name="tpsum", bufs=2, space="PSUM") as tpool,
        ):
            we_sb = cpool.tile([H, H], f16)
            nc.sync.dma_start(we_sb[:], we_in.ap())
            wohe_sb = cpool.tile([H, JROWS, JROWS], f16)
            nc.sync.dma_start(wohe_sb[:], wohe_in.ap())
            wp_sb = cpool.tile([H, P, H], f16)
            nc.sync.dma_start(wp_sb[:], wp_in.ap())
            wohn_sb = cpool.tile([H, JROWS, JROWS], f16)
            nc.sync.dma_start(wohn_sb[:], wohn_in.ap())

            ea_tiles = {}   # piece start -> tile
            na_tiles = {}
            eni, nni = 0, 0  # next piece index to DMA

            state = {"tacc": None, "tj": 0, "tgen": 0, "red": 0}
            pend = deque()   # [(psi, n_chunks, woh_sb)]

            def flush_reduce():
                psi, nch, woh = pend.popleft()
                for h in range(nch):
                    if state["tacc"] is None:
                        state["tacc"] = tpool.tile([JROWS, 512], f32,
                                                   name="tacc", tag="tacc")
                        state["tj"] = 0
                    tj = state["tj"]
                    last = (tj == JROWS - 1) or (state["red"] == N_RED - 1)
                    nc.tensor.matmul(
                        state["tacc"][:], woh[:, tj, :],
                        psi[:, h * 512:h * 512 + 512],
                        start=(tj == 0), stop=last,
                        skip_group_check=True,
                    )
                    state["tj"] += 1
                    state["red"] += 1
                    if last:
                        tsb = opool.tile([JROWS, 512], f32, tag="osb")
                        nc.scalar.copy(tsb[:], state["tacc"][:])
                        g = state["tgen"]
                        nc.sync.dma_start(
                            t_out.ap()[:, g * 512:g * 512 + 512], tsb[:])
                        state["tacc"] = None
                        state["tgen"] += 1

            def emit_edge_tile(k):
                nonlocal eni
                c0 = k * ZT
                while eni < len(epieces) and epieces[eni][0] < c0 + ZT:
                    p0, pw = epieces[eni]
                    pt = epool.tile([H, 4096], f16, tag="ea")
                    nc.sync.dma_start(pt[:, :pw], ea_in.ap()[:, p0:p0 + pw])
                    ea_tiles[p0] = pt
                    eni += 1
                z = zpool.tile([H, ZT], f32, tag="z")
                for h in range(ZT // 512):
                    c = c0 + h * 512
                    q0 = max(q for q in ea_tiles if q <= c)
                    pt = ea_tiles[q0]
                    nc.tensor.matmul(
                        z[:, h * 512:h * 512 + 512], we_sb[:],
                        pt[:, c - q0:c - q0 + 512],
                        start=True, stop=True,
                    )
                ex = xpool.tile([H, ZT], f16, tag="ex")
                nc.scalar.activation(ex[:], z[:], Exp)
                psi = ppool.tile([H, ZT], f16, tag="psi")
                nc.vector._custom_dve(elu_op, out=psi[:], in0=z[:],
                                      in1=ex[:], s0=1.0)
                pend.append((psi, ZT // 512, wohe_sb))

            def emit_node_tile(m):
                nonlocal nni
                c0 = m * P * 512
                while nni < len(npieces) and npieces[nni][0] < c0 + P * 512:
                    p0, pw = npieces[nni]
                    pt = npool.tile([H, 4096], f16, tag="na")
                    nc.sync.dma_start(pt[:, :pw], na_in.ap()[:, p0:p0 + pw])
                    na_tiles[p0] = pt
                    nni += 1
                z = zpool.tile([H, ZT], f32, tag="z")
                for p in range(P):
                    c = c0 + p * 512
                    q0 = max(q for q in na_tiles if q <= c)
                    pt = na_tiles[q0]
                    nc.tensor.matmul(
                        z[:, 0:512], wp_sb[:, p, :], pt[:, c - q0:c - q0 + 512],
                        start=(p == 0), stop=(p == P - 1),
                    )
                ex = xpool.tile([H, ZT], f16, tag="ex")
                nc.scalar.activation(ex[:, 0:512], z[:, 0:512], Exp)
                psi = ppool.tile([H, ZT], f16, tag="psi")
                nc.vector._custom_dve(elu_op, out=psi[:, 0:512],
                                      in0=z[:, 0:512], in1=ex[:, 0:512],
                                      s0=1.0)
                pend.append((psi, 1, wohn_sb))

            for kind, idx in tiles:
                if kind == "e":
                    emit_edge_tile(idx)
                else:
                    emit_node_tile(idx)
                if len(pend) > RED_DELAY:
                    flush_reduce()
            while pend:
                flush_reduce()

    nc.compile()
    return nc


# ---------------------------------------------------------------------------
def kernel(node_attrs, edge_attrs, instruction_batch, distribution,
           node_prop_similarities, relation_similarity,
           W_props, W_edge, w_node_score, w_rel_score,
           edge_indices, node_indices, edge_batch_indices):
    _install_ntff_hook()
    from concourse import bass_utils

    f16 = np.float16
    ea = np.asarray(edge_attrs, np.float32)
    na = np.asarray(node_attrs, np.float32)
    ib = np.asarray(instruction_batch, np.float32)
    dist = np.asarray(distribution, np.float32)
    nps = np.asarray(node_prop_similarities, np.float32)
    rs = np.asarray(relation_similarity, np.float32)
    Wp = np.asarray(W_props, np.float32)
    We = np.asarray(W_edge, np.float32)
    wn = np.asarray(w_node_score, np.float32)
    wr = np.asarray(w_rel_score, np.float32)
    ei = np.asarray(edge_indices).astype(np.int64)
    ni = np.asarray(node_indices).astype(np.int64)
    ebi = np.asarray(edge_batch_indices).astype(np.int64)
    src, dst = ei[0], ei[1]

    # ---- host prep: fold gatings into attrs, shard into equal slabs ----
    ea_t = np.zeros((NCORES, H, EC_PAD), f16)
    for c in range(NCORES):
        sl = slice(c * EC, (c + 1) * EC)
        ap = ea[sl] * ib[ebi[sl]]                      # (EC, H) f32
        ea_t[c, :, :EC] = ap.astype(f16).T

    na_t = np.zeros((NCORES, H, NT * P * 512), f16)
    for c in range(NCORES):
        sl = slice(c * NC, (c + 1) * NC)
        gate = nps[ni[sl]][:, :, None] * ib[ni[sl]][:, None, :]  # (NC,P,H)
        buf = np.zeros((NT * 512, P, H), f16)
        buf[:NC] = (na[sl] * gate).astype(f16)
        # (NT,512,P,H) -> (H, NT, P, 512) -> (H, NT*P*512)
        na_t[c] = np.ascontiguousarray(
            buf.reshape(NT, 512, P, H).transpose(3, 0, 2, 1)
        ).reshape(H, NT * P * 512)

    we_t = We.astype(f16)                              # (H[h], H[k])
    wp_t = np.ascontiguousarray(Wp.transpose(1, 0, 2)).astype(f16)
    woh_e = np.zeros((H, JROWS, JROWS), f16)
    woh_e[:, np.arange(JROWS), np.arange(JROWS)] = wr[:, None]
    woh_n = np.zeros((H, JROWS, JROWS), f16)
    woh_n[:, np.arange(JROWS), np.arange(JROWS)] = wn[:, None]

    nc = _build_program()

    in_maps = []
    for c in range(NCORES):
        in_maps.append({
            "ea_t": ea_t[c],
            "na_t": na_t[c],
            "we_t": we_t,
            "wp_t": wp_t,
            "woh_e": woh_e,
            "woh_n": woh_n,
        })

    res = bass_utils.run_bass_kernel_spmd(
        nc, in_maps, core_ids=list(range(NCORES)),
        trace=bool(os.environ.get("BASS_TRACE")),
        tmpdir=os.environ.get("KERNEL_TRACE_DIR") or None,
    )
    kernel.last_results = res  # for test.py profiling introspection

    # ---- host epilogue: un-weave the unified reduce stream ----
    _, chunks = _schedule()
    t_full = np.empty(E, np.float64)
    s_full = np.empty(N, np.float64)
    for c in range(NCORES):
        td = np.asarray(res.results[c]["t_out"], np.float64)  # (32, TG*512)
        for q, (kind, idx) in enumerate(chunks):
            vals = td[q % JROWS, (q // JROWS) * 512:(q // JROWS) * 512 + 512]
            if kind == "e":
                e0 = idx * 512
                w = min(512, EC - e0)
                if w > 0:
                    t_full[c * EC + e0:c * EC + e0 + w] = vals[:w]
            else:
                n0 = idx * 512
                w = min(512, NC - n0)
                if w > 0:
                    s_full[c * NC + n0:c * NC + n0 + w] = vals[:w]

    acc = np.bincount(dst, weights=dist[src].astype(np.float64) * t_full,
                      minlength=N)

    def seg_softmax(x):
        m = np.full(B, -np.inf)
        np.maximum.at(m, ni, x)
        e = np.exp(x - m[ni])
        ssum = np.zeros(B, np.float64)
        np.add.at(ssum, ni, e)
        return e / ssum[ni]

    next_rel = seg_softmax(acc)
    next_states = seg_softmax(s_full)
    rsn = rs[ni].astype(np.float64)
    out = rsn * next_rel + (1.0 - rsn) * next_states
    return out.astype(np.float32)


# revision 8
# speedup vs baseline: 1.2523x; 1.2250x over previous
"""NSMCell message-passing kernel for 8 Trainium2 NeuronCores.

Contract: kernel(**inputs) takes the FULL unsharded inputs (numpy/jax arrays)
and returns the FULL (N,) float32 output, matching reference.reference().

Math restructuring (exact, up to float assoc.):
  edge path:  t_e = w_rel . elu((i_b (*) a_e) @ W_edge),  b = edge_batch[e].
              Fold the gating into the attrs on host: a'_e = i_b (*) a_e,
              so ONE global stationary W_edge serves every edge - no graph
              boundaries on device, no edge sorting, cores take equal slabs.
  node path:  s_n = w_node . elu(sum_p (sim_bp * i_b (*) attr_np) @ W_props[p])
              with the (sim*i) gating likewise folded into attrs on host.
  host epilogue (O(N+E) scalar work): scatter-add dist[src]*t into nodes by
  dst, two segment softmaxes over graphs, final mix by relation_similarity.

Device pipeline per 1024-col z-tile (cols = edges or nodes, H=128 on
partitions):
  PE   : z = W^T @ a'            (2x512-col fp16 mains, f32 PSUM)
  ACT  : e = exp(z)              (fp16; saturates to inf for z>11, handled)
  DVE  : psi = relu(z) + min(e,1) - 1 = elu(z)   (one fused 4-op custom op)
  PE   : reduce matmul (emitted 2 tiles late so the PE never stalls on DVE):
         stationary = one-hot column j carrying w_rel (or w_node for node
         chunks); accumulates row j of a (32,512) PSUM bank. Edge and node
         chunks share ONE unified reduce stream / bank set, so PSUM fits
         3 z-buffers (6 banks) + 2 t-banks = 8 exactly.
The w.elu dot therefore costs 1 matmul per 512 cols instead of the per-128
LoadStationary matvec storm, and no per-graph weight tables are streamed.
_schedule() replays the emission order on the host to un-weave t_out.
"""

import os
import sys
import types
from collections import deque

import numpy as np

# ---------------------------------------------------------------------------
# problem constants (hardcoded per contract)
N, P, H, E, B = 100000, 4, 128, 1000000, 64
NCORES = 8
EC = E // NCORES            # 125000 edges per core (exact equal slabs)
NC = N // NCORES            # 12500 nodes per core
ZT = 1024                   # edge z-tile cols (2 PSUM banks)
EC_PAD = (EC + ZT - 1) // ZT * ZT          # 125952
N_ETILES = EC_PAD // ZT                    # 123
NT = (NC + 511) // 512                     # 25 node tiles of 512
JROWS = 8                   # t-accumulator PSUM rows (one-hot stationary set;
                            # few rows = little accumulate-RMW PSUM traffic)
N_RED = EC_PAD // 512 + NT  # 271 reduce chunks (edge + node unified stream)
TG = (N_RED + JROWS - 1) // JROWS          # 9 t-bank generations
RED_DELAY = 3               # emit a tile's reduces three tiles later


def _schedule():
    """Replay of the build-time emission order.

    Returns (tiles, chunks): tiles = list of ("e", k) / ("n", m) in tile
    emission order; chunks = list of ("e", edge_512_block) / ("n", m) in
    reduce-chunk order (the order chunks enter the shared t-bank stream)."""
    tiles = []
    node_next = 0
    for k in range(N_ETILES):
        tiles.append(("e", k))
        m_target = (k + 1) * NT // N_ETILES
        while node_next < min(m_target, NT):
            tiles.append(("n", node_next))
            node_next += 1
    while node_next < NT:
        tiles.append(("n", node_next))
        node_next += 1
    chunks = []
    pend = deque()
    for kind, idx in tiles:
        if kind == "e":
            pend.append([("e", idx * 2), ("e", idx * 2 + 1)])
        else:
            pend.append([("n", idx)])
        if len(pend) > RED_DELAY:
            chunks.extend(pend.popleft())
    while pend:
        chunks.extend(pend.popleft())
    assert len(chunks) == N_RED
    return tiles, chunks


# ---------------------------------------------------------------------------
def _install_ntff_hook():
    """Allow BASS_TRACE=1 profiling under axon (test.py); harmless otherwise."""
    try:
        from antenv.axon_hooks import get_axon_ntff_profile_hook  # noqa: F401
        return
    except ImportError:
        pass
    try:
        from trn_agent_boot.trn_boot import _ntff_profile_via_ctypes
        hook = _ntff_profile_via_ctypes("/opt/axon/libaxon_pjrt.so")
    except Exception:
        hook = None
    mod = types.ModuleType("antenv.axon_hooks")
    _state = {"hook": hook}
    mod.get_axon_ntff_profile_hook = lambda: _state["hook"]
    mod.set_axon_ntff_profile_hook = lambda h: _state.__setitem__("hook", h)
    sys.modules["antenv.axon_hooks"] = mod
    try:
        import antenv
        antenv.axon_hooks = mod
    except ImportError:
        pass


def _make_elu_op():
    """Register custom DVE op: out = s0 * (relu(in0) + min(in1, 1) - 1)
    (= s0 * elu(in0) when in1 == exp(in0)).
    Runtime registration: append to dve_ops.OPS."""
    from concourse import dve_ops
    from concourse.dve_spec import (Spec, Src0, Src1, C0, One, relu, minn,
                                    lower)
    from concourse.dve_uop import DveOpSpec

    name = "WELU_FROM_EXP_ANT"
    for op in dve_ops.OPS:
        if op.name == name:
            return op
    spec = Spec(
        body=(relu(Src0) + minn(Src1, One) - One) * C0,
        reference=lambda in0, in1, s0, s1, imm2: (
            (np.maximum(np.nan_to_num(in0, nan=0.0), 0)
             + np.minimum(in1, np.float32(1.0))
             - np.float32(1.0)) * s0
        ).astype(np.float32),
    )
    row = dve_ops._CUSTOM_DVE_ROW_BASE + len(dve_ops.OPS)
    assert row < 0x20
    shas = {}
    for ver in ("v3", "v4"):
        shas[ver] = DveOpSpec(
            name=name, opcode=row, uops=lower(spec, ver=ver), rd1_en=True
        ).sha(ver)
    op = dve_ops.DveOp(name, spec, subdim=False, uops_sha=shas)
    dve_ops.OPS.append(op)
    dve_ops.CUSTOM_DVE_SPECS[name] = spec
    dve_ops._SUB_OPCODE_FOR_NAME[name] = row
    return op


# ---------------------------------------------------------------------------
def _build_program():
    """Single SPMD bass program; every core runs an identical flat stream."""
    import concourse.tile as tile
    from concourse import bacc
    import concourse.mybir as mybir

    f32 = mybir.dt.float32
    f16 = mybir.dt.float16
    Exp = mybir.ActivationFunctionType.Exp
    elu_op = _make_elu_op()

    nc = bacc.Bacc("TRN2", target_bir_lowering=False, debug=False,
                   num_devices=NCORES)

    ea_in = nc.dram_tensor("ea_t", [H, EC_PAD], f16, kind="ExternalInput")
    na_in = nc.dram_tensor("na_t", [H, NT * P * 512], f16,
                           kind="ExternalInput")
    we_in = nc.dram_tensor("we_t", [H, H], f16, kind="ExternalInput")
    wp_in = nc.dram_tensor("wp_t", [H, P, H], f16, kind="ExternalInput")
    wohe_in = nc.dram_tensor("woh_e", [H, JROWS, JROWS], f16,
                             kind="ExternalInput")
    wohn_in = nc.dram_tensor("woh_n", [H, JROWS, JROWS], f16,
                             kind="ExternalInput")
    # t_out[j, g*512 + c] = value of reduce chunk (g*32 + j), col c; chunk
    # order is _schedule()[1] (mixed edge/node blocks of 512)
    t_out = nc.dram_tensor("t_out", [JROWS, TG * 512], f32,
                           kind="ExternalOutput")

    # edge DMA pieces (col ranges); first piece split for a fast launch
    epieces = [(0, 1024), (1024, 3072)]
    off = 4096
    while off < EC_PAD:
        w = min(4096, EC_PAD - off)
        epieces.append((off, w))
        off += w
    # node DMA pieces over the NT*P*512 column stream
    npieces = []
    off = 0
    ncols_total = NT * P * 512
    while off < ncols_total:
        w = min(4096, ncols_total - off)
        npieces.append((off, w))
        off += w

    tiles, _ = _schedule()

    with tile.TileContext(nc) as tc:
        with (
            tc.tile_pool(name="consts", bufs=1) as cpool,
            tc.tile_pool(name="ework", bufs=6) as epool,
            tc.tile_pool(name="nwork", bufs=3) as npool,
            tc.tile_pool(name="expw", bufs=4) as xpool,
            tc.tile_pool(name="psi", bufs=5) as ppool,
            tc.tile_pool(name="outs", bufs=2) as opool,
            tc.tile_pool(name="zpsum", bufs=3, space="PSUM") as zpool,
            tc.tile_pool(name="tpsum", bufs=2, space="PSUM") as tpool,
        ):
            we_sb = cpool.tile([H, H], f16)
            nc.sync.dma_start(we_sb[:], we_in.ap())
            wohe_sb = cpool.tile([H, JROWS, JROWS], f16)
            nc.sync.dma_start(wohe_sb[:], wohe_in.ap())
            wp_sb = cpool.tile([H, P, H], f16)
            nc.sync.dma_start(wp_sb[:], wp_in.ap())
            wohn_sb = cpool.tile([H, JROWS, JROWS], f16)
            nc.sync.dma_start(wohn_sb[:], wohn_in.ap())

            ea_tiles = {}   # piece start -> tile
            na_tiles = {}
            eni, nni = 0, 0  # next piece index to DMA

            state = {"tacc": None, "tj": 0, "tgen": 0, "red": 0}
            pend = deque()   # [(psi, n_chunks, woh_sb)]

            def flush_reduce():
                psi, nch, woh = pend.popleft()
                for h in range(nch):
                    if state["tacc"] is None:
                        state["tacc"] = tpool.tile([JROWS, 512], f32,
                                                   name="tacc", tag="tacc")
                        state["tj"] = 0
                    tj = state["tj"]
                    last = (tj == JROWS - 1) or (state["red"] == N_RED - 1)
                    nc.tensor.matmul(
                        state["tacc"][:], woh[:, tj, :],
                        psi[:, h * 512:h * 512 + 512],
                        start=(tj == 0), stop=last,
                        skip_group_check=True,
                    )
                    state["tj"] += 1
                    state["red"] += 1
                    if last:
                        tsb = opool.tile([JROWS, 512], f32, tag="osb")
                        nc.scalar.copy(tsb[:], state["tacc"][:])
                        g = state["tgen"]
                        nc.sync.dma_start(
                            t_out.ap()[:, g * 512:g * 512 + 512], tsb[:])
                        state["tacc"] = None
                        state["tgen"] += 1

            def emit_edge_tile(k):
                nonlocal eni
                c0 = k * ZT
                while eni < len(epieces) and epieces[eni][0] < c0 + ZT:
                    p0, pw = epieces[eni]
                    pt = epool.tile([H, 4096], f16, tag="ea")
                    nc.sync.dma_start(pt[:, :pw], ea_in.ap()[:, p0:p0 + pw])
                    ea_tiles[p0] = pt
                    eni += 1
                z = zpool.tile([H, ZT], f32, tag="z")
                for h in range(ZT // 512):
                    c = c0 + h * 512
                    q0 = max(q for q in ea_tiles if q <= c)
                    pt = ea_tiles[q0]
                    nc.tensor.matmul(
                        z[:, h * 512:h * 512 + 512], we_sb[:],
                        pt[:, c - q0:c - q0 + 512],
                        start=True, stop=True,
                    )
                ex = xpool.tile([H, ZT], f16, tag="ex")
                nc.scalar.activation(ex[:], z[:], Exp)
                psi = ppool.tile([H, ZT], f16, tag="psi")
                nc.vector._custom_dve(elu_op, out=psi[:], in0=z[:],
                                      in1=ex[:], s0=1.0)
                pend.append((psi, ZT // 512, wohe_sb))

            def emit_node_tile(m):
                nonlocal nni
                c0 = m * P * 512
                while nni < len(npieces) and npieces[nni][0] < c0 + P * 512:
                    p0, pw = npieces[nni]
                    pt = npool.tile([H, 4096], f16, tag="na")
                    nc.sync.dma_start(pt[:, :pw], na_in.ap()[:, p0:p0 + pw])
                    na_tiles[p0] = pt
                    nni += 1
                z = zpool.tile([H, ZT], f32, tag="z")
                for p in range(P):
                    c = c0 + p * 512
                    q0 = max(q for q in na_tiles if q <= c)
                    pt = na_tiles[q0]
                    nc.tensor.matmul(
                        z[:, 0:512], wp_sb[:, p, :], pt[:, c - q0:c - q0 + 512],
                        start=(p == 0), stop=(p == P - 1),
                    )
                ex = xpool.tile([H, ZT], f16, tag="ex")
                nc.scalar.activation(ex[:, 0:512], z[:, 0:512], Exp)
                psi = ppool.tile([H, ZT], f16, tag="psi")
                nc.vector._custom_dve(elu_op, out=psi[:, 0:512],
                                      in0=z[:, 0:512], in1=ex[:, 0:512],
                                      s0=1.0)
                pend.append((psi, 1, wohn_sb))

            for kind, idx in tiles:
                if kind == "e":
                    emit_edge_tile(idx)
                else:
                    emit_node_tile(idx)
                if len(pend) > RED_DELAY:
                    flush_reduce()
            while pend:
                flush_reduce()

            if os.environ.get("KERNEL_DIAG") == "1":
                # isolated-rate probes: back-to-back ACT/DVE ops with the
                # pipeline drained, to read inherent per-op durations
                zx = zpool.tile([H, ZT], f32, tag="z")
                pt = ea_tiles[max(ea_tiles)]
                for h in range(2):
                    nc.tensor.matmul(zx[:, h * 512:h * 512 + 512], we_sb[:],
                                     pt[:, h * 512:h * 512 + 512],
                                     start=True, stop=True)
                exd = [xpool.tile([H, ZT], f16, name=f"exd{i}", tag="ex")
                       for i in range(3)]
                for x in exd:
                    nc.scalar.activation(x[:], zx[:], Exp)
                for i in range(3):
                    ps = ppool.tile([H, ZT], f16, tag="psi")
                    nc.vector._custom_dve(elu_op, out=ps[:], in0=zx[:],
                                          in1=exd[i % 3][:], s0=1.0)

    nc.compile()
    return nc


# ---------------------------------------------------------------------------
def kernel(node_attrs, edge_attrs, instruction_batch, distribution,
           node_prop_similarities, relation_similarity,
           W_props, W_edge, w_node_score, w_rel_score,
           edge_indices, node_indices, edge_batch_indices):
    _install_ntff_hook()
    from concourse import bass_utils

    f16 = np.float16
    ea = np.asarray(edge_attrs, np.float32)
    na = np.asarray(node_attrs, np.float32)
    ib = np.asarray(instruction_batch, np.float32)
    dist = np.asarray(distribution, np.float32)
    nps = np.asarray(node_prop_similarities, np.float32)
    rs = np.asarray(relation_similarity, np.float32)
    Wp = np.asarray(W_props, np.float32)
    We = np.asarray(W_edge, np.float32)
    wn = np.asarray(w_node_score, np.float32)
    wr = np.asarray(w_rel_score, np.float32)
    ei = np.asarray(edge_indices).astype(np.int64)
    ni = np.asarray(node_indices).astype(np.int64)
    ebi = np.asarray(edge_batch_indices).astype(np.int64)
    src, dst = ei[0], ei[1]

    # ---- host prep: fold gatings into attrs, shard into equal slabs ----
    ea_t = np.zeros((NCORES, H, EC_PAD), f16)
    for c in range(NCORES):
        sl = slice(c * EC, (c + 1) * EC)
        ap = ea[sl] * ib[ebi[sl]]                      # (EC, H) f32
        ea_t[c, :, :EC] = ap.astype(f16).T

    na_t = np.zeros((NCORES, H, NT * P * 512), f16)
    for c in range(NCORES):
        sl = slice(c * NC, (c + 1) * NC)
        gate = nps[ni[sl]][:, :, None] * ib[ni[sl]][:, None, :]  # (NC,P,H)
        buf = np.zeros((NT * 512, P, H), f16)
        buf[:NC] = (na[sl] * gate).astype(f16)
        # (NT,512,P,H) -> (H, NT, P, 512) -> (H, NT*P*512)
        na_t[c] = np.ascontiguousarray(
            buf.reshape(NT, 512, P, H).transpose(3, 0, 2, 1)
        ).reshape(H, NT * P * 512)

    we_t = We.astype(f16)                              # (H[h], H[k])
    wp_t = np.ascontiguousarray(Wp.transpose(1, 0, 2)).astype(f16)
    woh_e = np.zeros((H, JROWS, JROWS), f16)
    woh_e[:, np.arange(JROWS), np.arange(JROWS)] = wr[:, None]
    woh_n = np.zeros((H, JROWS, JROWS), f16)
    woh_n[:, np.arange(JROWS), np.arange(JROWS)] = wn[:, None]

    nc = _build_program()

    in_maps = []
    for c in range(NCORES):
        in_maps.append({
            "ea_t": ea_t[c],
            "na_t": na_t[c],
            "we_t": we_t,
            "wp_t": wp_t,
            "woh_e": woh_e,
            "woh_n": woh_n,
        })

    res = bass_utils.run_bass_kernel_spmd(
        nc, in_maps, core_ids=list(range(NCORES)),
        trace=bool(os.environ.get("BASS_TRACE")),
        tmpdir=os.environ.get("KERNEL_TRACE_DIR") or None,
    )
    kernel.last_results = res  # for test.py profiling introspection

    # ---- host epilogue: un-weave the unified reduce stream ----
    _, chunks = _schedule()
    t_full = np.empty(E, np.float64)
    s_full = np.empty(N, np.float64)
    for c in range(NCORES):
        td = np.asarray(res.results[c]["t_out"], np.float64)  # (32, TG*512)
        for q, (kind, idx) in enumerate(chunks):
            vals = td[q % JROWS, (q // JROWS) * 512:(q // JROWS) * 512 + 512]
            if kind == "e":
                e0 = idx * 512
                w = min(512, EC - e0)
                if w > 0:
                    t_full[c * EC + e0:c * EC + e0 + w] = vals[:w]
            else:
                n0 = idx * 512
                w = min(512, NC - n0)
                if w > 0:
                    s_full[c * NC + n0:c * NC + n0 + w] = vals[:w]

    acc = np.bincount(dst, weights=dist[src].astype(np.float64) * t_full,
                      minlength=N)

    def seg_softmax(x):
        m = np.full(B, -np.inf)
        np.maximum.at(m, ni, x)
        e = np.exp(x - m[ni])
        ssum = np.zeros(B, np.float64)
        np.add.at(ssum, ni, e)
        return e / ssum[ni]

    next_rel = seg_softmax(acc)
    next_states = seg_softmax(s_full)
    rsn = rs[ni].astype(np.float64)
    out = rsn * next_rel + (1.0 - rsn) * next_states
    return out.astype(np.float32)
